# revision 16
# baseline (speedup 1.0000x reference)
"""Trainium2 Bass kernel for EntropySamplLoss, v12 (transposed PE-reduce).

Reference semantics (per image b):
  acts [N, P=320] viewed as [N, S=4, C=8, K=10] prototype groups
  ent[n, s, c] = normalized softmax entropy over the K protos of group (s, c)
  loss = mean over present (b, s, c) of (sum over pixels of class c of
         ent[n, s, c]) / count(c)

Layout (one image per NeuronCore, fp16 end-to-end):
  Host transposes acts to proto-major [640, M=N/2] fp16: row R = q*320 + P
  holds proto P of pixels with parity q (n = 2m + q), seen as 5 row-tiles
  of 128.  With protos on partitions the K=10 group sums become
  partition-axis reductions -> Tensor engine matmuls with fixed 0/1
  membership matrices gm[t] [128, 64] (PSUM row g = q*32 + s*8 + c):

    per pair-of-column-pairs (one [128, 1024] 2-bank Z PSUM tile + two
    [128, 512] U banks; each 512-col subchunk stacks two column blocks
    into the 128 PSUM rows):
      Z[g, m] = sum_t gm[t]^T @ exp(x_t)[:, m]      (PSUM accum, 5 matmuls)
      U[g, m] = sum_t gm[t]^T @ (x*exp(x))[:, m]    (5 matmuls)
      lnZ = Ln(Z)               (ACT, PSUM -> SBUF fp16, 1024 wide)
      rZ  = reciprocal_approx_fast(Z)               (DVE, 1024 wide)
      UrZ = U * rZ              (DVE scalar_tensor_tensor, PSUM src)
      ent = lnZ - UrZ           (DVE tensor_tensor, fp16 2x)
      num[pair] = sum_m mask*ent  (DVE scalar_tensor_tensor accum_out)
    host: per-class means from num sums + label counts, final masked mean.

  Key points vs the v6 baseline (388.7us, ACT-bound on exp+silu passes):
  exp runs ONCE on ACT (x*e^x is a DVE 2x fp16 mult, no silu pass, no ACT
  table switches); fp16 inputs halve HBM traffic; the Tensor engine
  replaces the DVE tree-sums (640 matmuls, fixed-cost pipelined); narrower
  prologue chunks prime the pipeline.  Engine busy (measured v11 trace):
  ACT ~84%, DVE ~82%, PE ~63%.  Measured: v7 199.5us -> v12 195.6us,
  rel err 8.7e-6 (gate 2e-2).
  Dead ends measured on HW: tensor_tensor_reduce crashes the device
  (INTERNAL) - use scalar_tensor_tensor accum_out instead; int32-bitcast
  log trick runs ~2.6x slower than fp16 STT on DVE; GPSIMD tensor_tensor
  offload slows DVE ~12% via the shared SBUF port; [128, 5*BCW] mega
  instructions (one exp / one mult per chunk) cost ~38us of pipeline
  overlap.
"""

import sys

if "/opt/trn_rl_repo" not in sys.path:
    sys.path.insert(0, "/opt/trn_rl_repo")

from contextlib import ExitStack

import numpy as np

import concourse.bacc as bacc
import concourse.bass as bass
import concourse.tile as tile
from concourse import mybir
from concourse.bass_utils import run_bass_kernel_spmd

# Problem shape (hardcoded per spec)
B, N, PP = 8, 65536, 320
S, C, K = 4, 8, 10
NCORES = 8

M = N // 2              # 32768 columns (column = even/odd pixel pair)
NT = 5                  # 640 transposed rows = 5 tiles of 128
SUB = 512               # PSUM-bank subchunk (512 f32 = one 2KB bank)
NSUB = M // SUB         # 64
NPAIR = NSUB // 2       # 32 stacked pairs
BCW = 4096              # big-chunk columns per DMA round (1 MiB per tile)
NBC = M // BCW          # 8
PAIRS_PER_BC = BCW // (2 * SUB)  # 4
G = 64                  # PSUM rows per subchunk: q(2) x s(4) x c(8)

_CACHE = {}


def _patch_act_tables():
    """Keep exp+ln in one ACT table set so no table switches are emitted."""
    import concourse.hw_specs as hw_specs

    tabs = hw_specs.get_activation_tables("gen3")
    E = mybir.ActivationFunctionType.Exp
    L = mybir.ActivationFunctionType.Ln
    for name, funcs in tabs.items():
        if name != "natural_log_exp_and_others":
            funcs.discard(E)
            funcs.discard(L)


def _group_matrices():
    """gm[t][p, g] = 1 iff transposed row R=128t+p belongs to PSUM row g."""
    gms = np.zeros((NT, 128, G), dtype=np.float16)
    for t in range(NT):
        for p in range(128):
            R = 128 * t + p
            q, P = divmod(R, PP)
            g = q * 32 + (P // 80) * 8 + (P % 80) // 10
            gms[t, p, g] = 1.0
    return gms


def _build():
    if "nc" in _CACHE:
        return _CACHE["nc"]

    _patch_act_tables()
    f32 = mybir.dt.float32
    f16 = mybir.dt.float16
    nc = bacc.Bacc("TRN2", target_bir_lowering=False, debug=False, num_devices=NCORES)

    acts_t = nc.dram_tensor("acts_t", [NT, 128, M], f16, kind="ExternalInput").ap()
    maskh = nc.dram_tensor("maskh", [128, M // 2], f16, kind="ExternalInput").ap()
    gmat = nc.dram_tensor("gmat", [NT, 128, G], f16, kind="ExternalInput").ap()
    parts1_out = nc.dram_tensor("parts1", [128, NPAIR], f32, kind="ExternalOutput").ap()
    parts2_out = nc.dram_tensor("parts2", [128, NPAIR], f32, kind="ExternalOutput").ap()

    with tile.TileContext(nc) as tc:
        with ExitStack() as ctx:
            singles = ctx.enter_context(tc.tile_pool(name="singles", bufs=1))
            xpool = ctx.enter_context(tc.tile_pool(name="xpool", bufs=2))
            epool = ctx.enter_context(tc.tile_pool(name="epool", bufs=2))
            mpool = ctx.enter_context(tc.tile_pool(name="mpool", bufs=2))
            spool = ctx.enter_context(tc.tile_pool(name="spool", bufs=4))
            psum = ctx.enter_context(tc.tile_pool(name="psum", bufs=3, space="PSUM"))

            parts1 = singles.tile([128, NPAIR], f32)

            # Prologue chunks are narrower so the first PSUM/DVE work starts
            # before a full 4096-column round of DMA+exp has completed.
            chunks = []
            c = 0
            for w in [1024, 1024, 2048] + [BCW] * ((M - 8192) // BCW) + [2048, 1024, 1024]:
                chunks.append((c, w))
                c += w
            assert c == M
            gms = []

            for c0, cw in chunks:
                xs, es = [], []
                for t in range(NT):
                    x = xpool.tile([128, BCW], f16, tag=f"x{t}")
                    nc.sync.dma_start(
                        out=x[:, 0:cw], in_=acts_t[t][:, c0 : c0 + cw]
                    )
                    xs.append(x)
                mk = mpool.tile([128, BCW // 2], f16, tag="mk")
                nc.sync.dma_start(
                    out=mk[:, 0 : cw // 2], in_=maskh[:, c0 // 2 : c0 // 2 + cw // 2]
                )
                if not gms:
                    # emitted after the first chunk's input DMAs so the first
                    # exp is not queued behind these on the DMA engine
                    for t in range(NT):
                        gm = singles.tile([128, G], f16, name=f"gm{t}")
                        nc.sync.dma_start(out=gm[:], in_=gmat[t])
                        gms.append(gm)
                for t in range(NT):
                    e = epool.tile([128, BCW], f16, tag=f"e{t}")
                    nc.scalar.activation(
                        out=e[:, 0:cw],
                        in_=xs[t][:, 0:cw],
                        func=mybir.ActivationFunctionType.Exp,
                    )
                    es.append(e)
                for t in range(NT):
                    # x := x * e^x in place (the U-matmul moving tensor)
                    nc.vector.tensor_tensor(
                        xs[t][:, 0:cw],
                        xs[t][:, 0:cw],
                        es[t][:, 0:cw],
                        mybir.AluOpType.mult,
                    )

                npl = cw // (2 * SUB)  # pairs in this chunk
                for u2 in range((npl + 1) // 2):
                    nph = min(2, npl - 2 * u2)
                    # Z for up to two pairs in one [128, 1024] 2-bank tile so
                    # the Ln and reciprocal amortize their instruction overhead
                    zp = psum.tile([128, 2 * SUB], f32, tag="z", bufs=2)
                    ups = []
                    for ph in range(nph):
                        u = 2 * u2 + ph
                        up = psum.tile([128, SUB], f32, tag=f"u{ph}", bufs=2)
                        ups.append(up)
                        for blk in range(2):
                            lo = (2 * u + blk) * SUB
                            sl = slice(lo, lo + SUB)
                            zout = zp[64 * blk : 64 * blk + 64, ph * SUB : (ph + 1) * SUB]
                            uout = up[64 * blk : 64 * blk + 64, :]
                            for t in range(NT):
                                nc.tensor.matmul(
                                    out=zout,
                                    lhsT=gms[t][:],
                                    rhs=es[t][:, sl],
                                    start=(t == 0),
                                    stop=(t == NT - 1),
                                    skip_group_check=True,
                                )
                                nc.tensor.matmul(
                                    out=uout,
                                    lhsT=gms[t][:],
                                    rhs=xs[t][:, sl],
                                    start=(t == 0),
                                    stop=(t == NT - 1),
                                    skip_group_check=True,
                                )

                    zv = zp[:, 0 : nph * SUB]
                    lnz = spool.tile([128, 2 * SUB], f16, tag="lnz")
                    nc.scalar.activation(
                        out=lnz[:, 0 : nph * SUB],
                        in_=zv,
                        func=mybir.ActivationFunctionType.Ln,
                    )
                    rz = spool.tile([128, 2 * SUB], f32, tag="rz")
                    nc.vector.reciprocal_approx_fast(out=rz[:, 0 : nph * SUB], in_=zv)
                    for ph in range(nph):
                        u = 2 * u2 + ph
                        pair = c0 // (2 * SUB) + u
                        msl = mk[:, u * SUB : (u + 1) * SUB]
                        phs = slice(ph * SUB, (ph + 1) * SUB)
                        urz = spool.tile([128, SUB], f16, tag="urz")
                        nc.vector.scalar_tensor_tensor(
                            out=urz[:],
                            in0=ups[ph][:],
                            scalar=1.0,
                            in1=rz[:, phs],
                            op0=mybir.AluOpType.mult,
                            op1=mybir.AluOpType.mult,
                        )
                        # ent = lnZ - U/Z, then one masked accumulation
                        ent = spool.tile([128, SUB], f16, tag="ent")
                        nc.vector.tensor_tensor(
                            ent[:], lnz[:, phs], urz[:], mybir.AluOpType.subtract
                        )
                        d1 = spool.tile([128, SUB], f16, tag="d1")
                        nc.vector.scalar_tensor_tensor(
                            out=d1[:],
                            in0=ent[:],
                            scalar=1.0,
                            in1=msl,
                            op0=mybir.AluOpType.mult,
                            op1=mybir.AluOpType.mult,
                            accum_out=parts1[:, pair : pair + 1],
                        )

            nc.sync.dma_start(out=parts1_out, in_=parts1[:])
            nc.sync.dma_start(out=parts2_out, in_=parts1[:])

    nc.compile()
    _CACHE["nc"] = nc
    return nc


def _prep_inputs(prototype_activations, target_labels, proto_idx):
    acts = np.asarray(prototype_activations, dtype=np.float32)
    labels = np.asarray(target_labels)
    pidx = np.asarray(proto_idx)

    expected = np.arange(S * C * K, dtype=np.int64).reshape(S, C, K)
    if not np.array_equal(pidx.astype(np.int64), expected):
        # general (slow) fallback: permute proto columns on host
        acts = np.ascontiguousarray(acts[..., pidx.reshape(-1)])

    gms = _group_matrices()
    in_maps = []
    for b in range(B):
        x16 = acts[b].astype(np.float16)  # [N, 320]
        # [640, M]: row q*320+P = proto P of pixels n = 2m+q
        at = np.ascontiguousarray(
            x16.reshape(M, 2, PP).transpose(1, 2, 0)
        ).reshape(NT, 128, M)

        lab = labels[b].astype(np.int32)
        # L[q, u, blk, x] = label of pixel n = 2*(512*(2u+blk)+x) + q
        L = np.ascontiguousarray(lab.reshape(M, 2).T).reshape(2, NPAIR, 2, SUB)
        eq = L[:, :, :, :, None] == np.arange(1, C + 1, dtype=np.int32)
        # maskh[blk*64 + q*32 + s*8 + c, u*512 + x]
        mh = np.broadcast_to(
            eq.transpose(2, 0, 4, 1, 3)[:, :, None, :, :, :],
            (2, 2, S, C, NPAIR, SUB),
        ).astype(np.float16)
        in_maps.append(
            {
                "acts_t": at,
                "maskh": np.ascontiguousarray(mh).reshape(128, M // 2),
                "gmat": gms,
            }
        )
    return in_maps, labels


def _combine(parts_list, labels):
    """parts_list: per-core (parts1 [128, 32], parts2 [128, 32]) f32.
    Row = blk*64 + q*32 + s*8 + c, col = pair index."""
    num = np.zeros((B, S, C), dtype=np.float64)
    cnt = np.zeros((B, C), dtype=np.int64)
    for b, (p1, p2) in enumerate(parts_list):
        d = p1.astype(np.float64).sum(axis=1)
        num[b] = d.reshape(2, 2, S, C).sum(axis=(0, 1))
        lab = np.asarray(labels[b]).astype(np.int64)
        cnt[b] = np.bincount(lab, minlength=C + 1)[1 : C + 1]
    num /= np.log(K)
    present = cnt > 0
    mean_ent = num / np.maximum(cnt, 1)[:, None, :]
    n_entries = float(present.sum() * S)
    total = float((mean_ent * present[:, None, :]).sum())
    if n_entries > 0:
        return np.float32(total / max(n_entries, 1.0))
    return np.float32(0.0)


def kernel(prototype_activations, target_labels, proto_idx, _trace=False, _tmpdir=None):
    nc = _build()
    in_maps, labels = _prep_inputs(prototype_activations, target_labels, proto_idx)
    res = run_bass_kernel_spmd(
        nc, in_maps, list(range(NCORES)), trace=_trace, tmpdir=_tmpdir
    )
    parts_list = [
        (res.results[i]["parts1"], res.results[i]["parts2"]) for i in range(NCORES)
    ]
    out = _combine(parts_list, labels)
    if _trace:
        return out, res
    return out


# revision 19
# speedup vs baseline: 1.0210x; 1.0210x over previous
"""Trainium2 Bass kernel for EntropySamplLoss, v13 (transposed PE-reduce).

Reference semantics (per image b):
  acts [N, P=320] viewed as [N, S=4, C=8, K=10] prototype groups
  ent[n, s, c] = normalized softmax entropy over the K protos of group (s, c)
  loss = mean over present (b, s, c) of (sum over pixels of class c of
         ent[n, s, c]) / count(c)

Layout (one image per NeuronCore, fp16 end-to-end):
  Host transposes acts to proto-major [640, M=N/2] fp16: row R = q*320 + P
  holds proto P of pixels with parity q (n = 2m + q), seen as 5 row-tiles
  of 128.  With protos on partitions the K=10 group sums become
  partition-axis reductions -> Tensor engine matmuls with fixed 0/1
  membership matrices gm[t] [128, 64] (PSUM row g = q*32 + s*8 + c):

    per pair-of-column-pairs (one [128, 1024] 2-bank Z PSUM tile + two
    [128, 512] U banks; each 512-col subchunk stacks two column blocks
    into the 128 PSUM rows):
      Z[g, m] = sum_t gm[t]^T @ exp(x_t)[:, m]      (PSUM accum, 5 matmuls)
      U[g, m] = sum_t gm[t]^T @ (x*exp(x))[:, m]    (5 matmuls)
      lnZ = Ln(Z)               (ACT, PSUM -> SBUF fp16, 1024 wide)
      rZ  = reciprocal_approx_fast(Z)               (DVE, 1024 wide)
      UrZ = U * rZ              (DVE scalar_tensor_tensor, PSUM src)
      ent = lnZ - UrZ           (DVE tensor_tensor, fp16 2x)
      num[pair] = sum_m mask*ent  (DVE scalar_tensor_tensor accum_out)
    host: per-class means from num sums + label counts, final masked mean.

  Key points vs the v6 baseline (388.7us, ACT-bound on exp+silu passes):
  exp runs ONCE on ACT (x*e^x is a DVE 2x fp16 mult, no silu pass, no ACT
  table switches); fp16 inputs halve HBM traffic; the Tensor engine
  replaces the DVE tree-sums (640 matmuls, fixed-cost pipelined); narrower
  prologue/epilogue chunks (2048 cols) trim pipeline head/tail, and the
  gm weight loads are queued after the first chunk's input DMAs.  Engine
  busy mid-kernel: ACT 100%, DVE ~99% (both at their floor), PE ~75%.
  Measured: v7 199.5us -> v13 193.0us, rel err 8.7e-6 (gate 2e-2).
  Dead ends measured on HW: tensor_tensor_reduce crashes the device
  (INTERNAL) - use scalar_tensor_tensor accum_out instead; int32-bitcast
  log trick runs ~2.6x slower than fp16 STT on DVE; GPSIMD tensor_tensor
  offload slows DVE ~12% via the shared SBUF port; [128, 5*BCW] mega
  instructions (one exp / one mult per chunk) cost ~38us of pipeline
  overlap.
"""

import sys

if "/opt/trn_rl_repo" not in sys.path:
    sys.path.insert(0, "/opt/trn_rl_repo")

from contextlib import ExitStack

import numpy as np

import concourse.bacc as bacc
import concourse.bass as bass
import concourse.tile as tile
from concourse import mybir
from concourse.bass_utils import run_bass_kernel_spmd

# Problem shape (hardcoded per spec)
B, N, PP = 8, 65536, 320
S, C, K = 4, 8, 10
NCORES = 8

M = N // 2              # 32768 columns (column = even/odd pixel pair)
NT = 5                  # 640 transposed rows = 5 tiles of 128
SUB = 512               # PSUM-bank subchunk (512 f32 = one 2KB bank)
NSUB = M // SUB         # 64
NPAIR = NSUB // 2       # 32 stacked pairs
BCW = 4096              # big-chunk columns per DMA round (1 MiB per tile)
NBC = M // BCW          # 8
PAIRS_PER_BC = BCW // (2 * SUB)  # 4
G = 64                  # PSUM rows per subchunk: q(2) x s(4) x c(8)

_CACHE = {}


def _patch_act_tables():
    """Keep exp+ln in one ACT table set so no table switches are emitted."""
    import concourse.hw_specs as hw_specs

    tabs = hw_specs.get_activation_tables("gen3")
    E = mybir.ActivationFunctionType.Exp
    L = mybir.ActivationFunctionType.Ln
    for name, funcs in tabs.items():
        if name != "natural_log_exp_and_others":
            funcs.discard(E)
            funcs.discard(L)


def _group_matrices():
    """gm[t][p, g] = 1 iff transposed row R=128t+p belongs to PSUM row g."""
    gms = np.zeros((NT, 128, G), dtype=np.float16)
    for t in range(NT):
        for p in range(128):
            R = 128 * t + p
            q, P = divmod(R, PP)
            g = q * 32 + (P // 80) * 8 + (P % 80) // 10
            gms[t, p, g] = 1.0
    return gms


def _build():
    if "nc" in _CACHE:
        return _CACHE["nc"]

    _patch_act_tables()
    f32 = mybir.dt.float32
    f16 = mybir.dt.float16
    nc = bacc.Bacc("TRN2", target_bir_lowering=False, debug=False, num_devices=NCORES)

    acts_t = nc.dram_tensor("acts_t", [NT, 128, M], f16, kind="ExternalInput").ap()
    maskh = nc.dram_tensor("maskh", [128, M // 2], f16, kind="ExternalInput").ap()
    gmat = nc.dram_tensor("gmat", [NT, 128, G], f16, kind="ExternalInput").ap()
    parts1_out = nc.dram_tensor("parts1", [128, NPAIR], f32, kind="ExternalOutput").ap()
    parts2_out = nc.dram_tensor("parts2", [128, NPAIR], f32, kind="ExternalOutput").ap()

    with tile.TileContext(nc) as tc:
        with ExitStack() as ctx:
            singles = ctx.enter_context(tc.tile_pool(name="singles", bufs=1))
            xpool = ctx.enter_context(tc.tile_pool(name="xpool", bufs=2))
            epool = ctx.enter_context(tc.tile_pool(name="epool", bufs=2))
            mpool = ctx.enter_context(tc.tile_pool(name="mpool", bufs=2))
            spool = ctx.enter_context(tc.tile_pool(name="spool", bufs=4))
            psum = ctx.enter_context(tc.tile_pool(name="psum", bufs=3, space="PSUM"))

            parts1 = singles.tile([128, NPAIR], f32)

            # Prologue chunks are narrower so the first PSUM/DVE work starts
            # before a full 4096-column round of DMA+exp has completed.
            chunks = []
            c = 0
            for w in [2048, 2048] + [BCW] * ((M - 8192) // BCW) + [2048, 2048]:
                chunks.append((c, w))
                c += w
            assert c == M
            gms = []

            for ci, (c0, cw) in enumerate(chunks):
                # in the pipeline tail ACT is idle while DVE drains the last
                # pair-ops: compute 1/Z there as exp(-lnZ) on ACT instead
                act_rz = ci >= len(chunks) - 2
                xs, es = [], []
                for t in range(NT):
                    x = xpool.tile([128, BCW], f16, tag=f"x{t}")
                    nc.sync.dma_start(
                        out=x[:, 0:cw], in_=acts_t[t][:, c0 : c0 + cw]
                    )
                    xs.append(x)
                mk = mpool.tile([128, BCW // 2], f16, tag="mk")
                nc.sync.dma_start(
                    out=mk[:, 0 : cw // 2], in_=maskh[:, c0 // 2 : c0 // 2 + cw // 2]
                )
                if not gms:
                    # emitted after the first chunk's input DMAs so the first
                    # exp is not queued behind these on the DMA engine
                    for t in range(NT):
                        gm = singles.tile([128, G], f16, name=f"gm{t}")
                        nc.sync.dma_start(out=gm[:], in_=gmat[t])
                        gms.append(gm)
                for t in range(NT):
                    e = epool.tile([128, BCW], f16, tag=f"e{t}")
                    nc.scalar.activation(
                        out=e[:, 0:cw],
                        in_=xs[t][:, 0:cw],
                        func=mybir.ActivationFunctionType.Exp,
                    )
                    es.append(e)
                for t in range(NT):
                    # x := x * e^x in place (the U-matmul moving tensor)
                    nc.vector.tensor_tensor(
                        xs[t][:, 0:cw],
                        xs[t][:, 0:cw],
                        es[t][:, 0:cw],
                        mybir.AluOpType.mult,
                    )

                for u2 in range(cw // (4 * SUB)):
                    # Z for two pairs in one [128, 1024] 2-bank tile so the
                    # Ln and reciprocal amortize their instruction overhead
                    zp = psum.tile([128, 2 * SUB], f32, tag="z", bufs=2)
                    ups = []
                    for ph in range(2):
                        u = 2 * u2 + ph
                        up = psum.tile([128, SUB], f32, tag=f"u{ph}", bufs=2)
                        ups.append(up)
                        for blk in range(2):
                            lo = (2 * u + blk) * SUB
                            sl = slice(lo, lo + SUB)
                            zout = zp[64 * blk : 64 * blk + 64, ph * SUB : (ph + 1) * SUB]
                            uout = up[64 * blk : 64 * blk + 64, :]
                            for t in range(NT):
                                nc.tensor.matmul(
                                    out=zout,
                                    lhsT=gms[t][:],
                                    rhs=es[t][:, sl],
                                    start=(t == 0),
                                    stop=(t == NT - 1),
                                    skip_group_check=True,
                                )
                                nc.tensor.matmul(
                                    out=uout,
                                    lhsT=gms[t][:],
                                    rhs=xs[t][:, sl],
                                    start=(t == 0),
                                    stop=(t == NT - 1),
                                    skip_group_check=True,
                                )

                    lnz = spool.tile([128, 2 * SUB], f16, tag="lnz")
                    nc.scalar.activation(
                        out=lnz[:], in_=zp[:], func=mybir.ActivationFunctionType.Ln
                    )
                    rz = spool.tile([128, 2 * SUB], f32, tag="rz")
                    if act_rz:
                        nc.scalar.activation(
                            out=rz[:],
                            in_=lnz[:],
                            func=mybir.ActivationFunctionType.Exp,
                            scale=-1.0,
                        )
                    else:
                        nc.vector.reciprocal_approx_fast(out=rz[:], in_=zp[:])
                    for ph in range(2):
                        u = 2 * u2 + ph
                        pair = c0 // (2 * SUB) + u
                        msl = mk[:, u * SUB : (u + 1) * SUB]
                        phs = slice(ph * SUB, (ph + 1) * SUB)
                        urz = spool.tile([128, SUB], f16, tag="urz")
                        nc.vector.scalar_tensor_tensor(
                            out=urz[:],
                            in0=ups[ph][:],
                            scalar=1.0,
                            in1=rz[:, phs],
                            op0=mybir.AluOpType.mult,
                            op1=mybir.AluOpType.mult,
                        )
                        # ent = lnZ - U/Z, then one masked accumulation
                        ent = spool.tile([128, SUB], f16, tag="ent")
                        nc.vector.tensor_tensor(
                            ent[:], lnz[:, phs], urz[:], mybir.AluOpType.subtract
                        )
                        d1 = spool.tile([128, SUB], f16, tag="d1")
                        nc.vector.scalar_tensor_tensor(
                            out=d1[:],
                            in0=ent[:],
                            scalar=1.0,
                            in1=msl,
                            op0=mybir.AluOpType.mult,
                            op1=mybir.AluOpType.mult,
                            accum_out=parts1[:, pair : pair + 1],
                        )

            nc.sync.dma_start(out=parts1_out, in_=parts1[:])
            nc.sync.dma_start(out=parts2_out, in_=parts1[:])

    nc.compile()
    _CACHE["nc"] = nc
    return nc


def _prep_inputs(prototype_activations, target_labels, proto_idx):
    acts = np.asarray(prototype_activations, dtype=np.float32)
    labels = np.asarray(target_labels)
    pidx = np.asarray(proto_idx)

    expected = np.arange(S * C * K, dtype=np.int64).reshape(S, C, K)
    if not np.array_equal(pidx.astype(np.int64), expected):
        # general (slow) fallback: permute proto columns on host
        acts = np.ascontiguousarray(acts[..., pidx.reshape(-1)])

    gms = _group_matrices()
    in_maps = []
    for b in range(B):
        x16 = acts[b].astype(np.float16)  # [N, 320]
        # [640, M]: row q*320+P = proto P of pixels n = 2m+q
        at = np.ascontiguousarray(
            x16.reshape(M, 2, PP).transpose(1, 2, 0)
        ).reshape(NT, 128, M)

        lab = labels[b].astype(np.int32)
        # L[q, u, blk, x] = label of pixel n = 2*(512*(2u+blk)+x) + q
        L = np.ascontiguousarray(lab.reshape(M, 2).T).reshape(2, NPAIR, 2, SUB)
        eq = L[:, :, :, :, None] == np.arange(1, C + 1, dtype=np.int32)
        # maskh[blk*64 + q*32 + s*8 + c, u*512 + x]
        mh = np.broadcast_to(
            eq.transpose(2, 0, 4, 1, 3)[:, :, None, :, :, :],
            (2, 2, S, C, NPAIR, SUB),
        ).astype(np.float16)
        in_maps.append(
            {
                "acts_t": at,
                "maskh": np.ascontiguousarray(mh).reshape(128, M // 2),
                "gmat": gms,
            }
        )
    return in_maps, labels


def _combine(parts_list, labels):
    """parts_list: per-core (parts1 [128, 32], parts2 [128, 32]) f32.
    Row = blk*64 + q*32 + s*8 + c, col = pair index."""
    num = np.zeros((B, S, C), dtype=np.float64)
    cnt = np.zeros((B, C), dtype=np.int64)
    for b, (p1, p2) in enumerate(parts_list):
        d = p1.astype(np.float64).sum(axis=1)
        num[b] = d.reshape(2, 2, S, C).sum(axis=(0, 1))
        lab = np.asarray(labels[b]).astype(np.int64)
        cnt[b] = np.bincount(lab, minlength=C + 1)[1 : C + 1]
    num /= np.log(K)
    present = cnt > 0
    mean_ent = num / np.maximum(cnt, 1)[:, None, :]
    n_entries = float(present.sum() * S)
    total = float((mean_ent * present[:, None, :]).sum())
    if n_entries > 0:
        return np.float32(total / max(n_entries, 1.0))
    return np.float32(0.0)


def kernel(prototype_activations, target_labels, proto_idx, _trace=False, _tmpdir=None):
    nc = _build()
    in_maps, labels = _prep_inputs(prototype_activations, target_labels, proto_idx)
    res = run_bass_kernel_spmd(
        nc, in_maps, list(range(NCORES)), trace=_trace, tmpdir=_tmpdir
    )
    parts_list = [
        (res.results[i]["parts1"], res.results[i]["parts2"]) for i in range(NCORES)
    ]
    out = _combine(parts_list, labels)
    if _trace:
        return out, res
    return out


# revision 22
# speedup vs baseline: 1.0361x; 1.0148x over previous
"""Trainium2 Bass kernel for EntropySamplLoss, v15 (transposed PE-reduce).

Reference semantics (per image b):
  acts [N, P=320] viewed as [N, S=4, C=8, K=10] prototype groups
  ent[n, s, c] = normalized softmax entropy over the K protos of group (s, c)
  loss = mean over present (b, s, c) of (sum over pixels of class c of
         ent[n, s, c]) / count(c)

Layout (one image per NeuronCore, fp16 end-to-end):
  Host transposes acts to proto-major [640, M=N/2] fp16: row R = q*320 + P
  holds proto P of pixels with parity q (n = 2m + q), seen as 5 row-tiles
  of 128.  With protos on partitions the K=10 group sums become
  partition-axis reductions -> Tensor engine matmuls with fixed 0/1
  membership matrices gm[t] [128, 64] (PSUM row g = q*32 + s*8 + c):

    per pair-of-column-pairs (one [128, 1024] 2-bank Z PSUM tile + two
    [128, 512] U banks; each 512-col subchunk stacks two column blocks
    into the 128 PSUM rows):
      Z[g, m] = sum_t gm[t]^T @ exp(x_t)[:, m]      (PSUM accum, 5 matmuls)
      U[g, m] = sum_t gm[t]^T @ (x*exp(x))[:, m]    (5 matmuls)
      lnZ = Ln(Z)               (ACT, PSUM -> SBUF fp16, 1024 wide)
      rZ  = reciprocal_approx_fast(Z)               (DVE, 1024 wide)
      UrZ = U * rZ              (DVE scalar_tensor_tensor, PSUM src)
      ent = lnZ - UrZ           (DVE tensor_tensor, fp16 2x)
      num[pair] = sum_m mask*ent  (DVE scalar_tensor_tensor accum_out)
    host: per-class means from num sums + label counts, final masked mean.

  Key points vs the v6 baseline (388.7us, ACT-bound on exp+silu passes):
  exp runs ONCE on ACT (x*e^x is a DVE 2x fp16 mult, no silu pass, no ACT
  table switches); fp16 inputs halve HBM traffic; the Tensor engine
  replaces the DVE tree-sums (640 matmuls, fixed-cost pipelined); narrower
  prologue/epilogue chunks (2048 cols) trim pipeline head/tail, and the
  gm weight loads are queued after the first chunk's input DMAs.  Engine
  busy mid-kernel: ACT 100%, DVE ~99% (both at their floor), PE ~75%.
  Measured: v7 199.5us -> v15 192.7us (tail quads compute
  1/Z as exp(-lnZ) on the then-idle ACT instead of the DVE), rel err 8.7e-6 (gate 2e-2).
  Dead ends measured on HW: tensor_tensor_reduce crashes the device
  (INTERNAL) - use scalar_tensor_tensor accum_out instead; int32-bitcast
  log trick runs ~2.6x slower than fp16 STT on DVE; GPSIMD tensor_tensor
  offload slows DVE ~12% via the shared SBUF port; [128, 5*BCW] mega
  instructions (one exp / one mult per chunk) cost ~38us of pipeline
  overlap.
"""

import sys

if "/opt/trn_rl_repo" not in sys.path:
    sys.path.insert(0, "/opt/trn_rl_repo")

from contextlib import ExitStack

import numpy as np

import concourse.bacc as bacc
import concourse.bass as bass
import concourse.tile as tile
from concourse import mybir
from concourse.bass_utils import run_bass_kernel_spmd

# Problem shape (hardcoded per spec)
B, N, PP = 8, 65536, 320
S, C, K = 4, 8, 10
NCORES = 8

M = N // 2              # 32768 columns (column = even/odd pixel pair)
NT = 5                  # 640 transposed rows = 5 tiles of 128
SUB = 512               # PSUM-bank subchunk (512 f32 = one 2KB bank)
NSUB = M // SUB         # 64
NPAIR = NSUB // 2       # 32 stacked pairs
BCW = 4096              # big-chunk columns per DMA round (1 MiB per tile)
NBC = M // BCW          # 8
PAIRS_PER_BC = BCW // (2 * SUB)  # 4
G = 64                  # PSUM rows per subchunk: q(2) x s(4) x c(8)

_CACHE = {}


def _patch_act_tables():
    """Keep exp+ln in one ACT table set so no table switches are emitted."""
    import concourse.hw_specs as hw_specs

    tabs = hw_specs.get_activation_tables("gen3")
    E = mybir.ActivationFunctionType.Exp
    L = mybir.ActivationFunctionType.Ln
    for name, funcs in tabs.items():
        if name != "natural_log_exp_and_others":
            funcs.discard(E)
            funcs.discard(L)


def _group_matrices():
    """gm[t][p, g] = 1 iff transposed row R=128t+p belongs to PSUM row g."""
    gms = np.zeros((NT, 128, G), dtype=np.float16)
    for t in range(NT):
        for p in range(128):
            R = 128 * t + p
            q, P = divmod(R, PP)
            g = q * 32 + (P // 80) * 8 + (P % 80) // 10
            gms[t, p, g] = 1.0
    return gms


def _build():
    if "nc" in _CACHE:
        return _CACHE["nc"]

    _patch_act_tables()
    f32 = mybir.dt.float32
    f16 = mybir.dt.float16
    nc = bacc.Bacc("TRN2", target_bir_lowering=False, debug=False, num_devices=NCORES)

    acts_t = nc.dram_tensor("acts_t", [NT, 128, M], f16, kind="ExternalInput").ap()
    maskh = nc.dram_tensor("maskh", [128, M // 2], f16, kind="ExternalInput").ap()
    gmat = nc.dram_tensor("gmat", [NT, 128, G], f16, kind="ExternalInput").ap()
    parts1_out = nc.dram_tensor("parts1", [128, NPAIR], f32, kind="ExternalOutput").ap()
    parts2_out = nc.dram_tensor("parts2", [128, NPAIR], f32, kind="ExternalOutput").ap()

    with tile.TileContext(nc) as tc:
        with ExitStack() as ctx:
            singles = ctx.enter_context(tc.tile_pool(name="singles", bufs=1))
            xpool = ctx.enter_context(tc.tile_pool(name="xpool", bufs=2))
            epool = ctx.enter_context(tc.tile_pool(name="epool", bufs=2))
            mpool = ctx.enter_context(tc.tile_pool(name="mpool", bufs=2))
            spool = ctx.enter_context(tc.tile_pool(name="spool", bufs=4))
            psum = ctx.enter_context(tc.tile_pool(name="psum", bufs=3, space="PSUM"))

            parts1 = singles.tile([128, NPAIR], f32)

            # Prologue chunks are narrower so the first PSUM/DVE work starts
            # before a full 4096-column round of DMA+exp has completed.
            chunks = []
            c = 0
            for w in [2048, 2048] + [BCW] * ((M - 8192) // BCW) + [2048, 2048]:
                chunks.append((c, w))
                c += w
            assert c == M
            gms = []

            for ci, (c0, cw) in enumerate(chunks):
                # in the pipeline tail ACT is idle while DVE drains the last
                # pair-ops: compute 1/Z there as exp(-lnZ) on ACT instead
                act_rz = ci >= len(chunks) - 2
                xs, es = [], []
                for t in range(NT):
                    x = xpool.tile([128, BCW], f16, tag=f"x{t}")
                    nc.sync.dma_start(
                        out=x[:, 0:cw], in_=acts_t[t][:, c0 : c0 + cw]
                    )
                    xs.append(x)
                mk = mpool.tile([128, BCW // 2], f16, tag="mk")
                nc.sync.dma_start(
                    out=mk[:, 0 : cw // 2], in_=maskh[:, c0 // 2 : c0 // 2 + cw // 2]
                )
                if not gms:
                    # emitted after the first chunk's input DMAs so the first
                    # exp is not queued behind these on the DMA engine
                    for t in range(NT):
                        gm = singles.tile([128, G], f16, name=f"gm{t}")
                        nc.sync.dma_start(out=gm[:], in_=gmat[t])
                        gms.append(gm)
                for t in range(NT):
                    e = epool.tile([128, BCW], f16, tag=f"e{t}")
                    nc.scalar.activation(
                        out=e[:, 0:cw],
                        in_=xs[t][:, 0:cw],
                        func=mybir.ActivationFunctionType.Exp,
                    )
                    es.append(e)
                for t in range(NT):
                    # x := x * e^x in place (the U-matmul moving tensor)
                    nc.vector.tensor_tensor(
                        xs[t][:, 0:cw],
                        xs[t][:, 0:cw],
                        es[t][:, 0:cw],
                        mybir.AluOpType.mult,
                    )

                for u2 in range(cw // (4 * SUB)):
                    # Z for two pairs in one [128, 1024] 2-bank tile so the
                    # Ln and reciprocal amortize their instruction overhead
                    zp = psum.tile([128, 2 * SUB], f32, tag="z", bufs=2)
                    ups = []
                    for ph in range(2):
                        u = 2 * u2 + ph
                        up = psum.tile([128, SUB], f32, tag=f"u{ph}", bufs=2)
                        ups.append(up)
                        for blk in range(2):
                            lo = (2 * u + blk) * SUB
                            sl = slice(lo, lo + SUB)
                            zout = zp[64 * blk : 64 * blk + 64, ph * SUB : (ph + 1) * SUB]
                            uout = up[64 * blk : 64 * blk + 64, :]
                            for t in range(NT):
                                nc.tensor.matmul(
                                    out=zout,
                                    lhsT=gms[t][:],
                                    rhs=es[t][:, sl],
                                    start=(t == 0),
                                    stop=(t == NT - 1),
                                    skip_group_check=True,
                                )
                                nc.tensor.matmul(
                                    out=uout,
                                    lhsT=gms[t][:],
                                    rhs=xs[t][:, sl],
                                    start=(t == 0),
                                    stop=(t == NT - 1),
                                    skip_group_check=True,
                                )

                    lnz = spool.tile([128, 2 * SUB], f16, tag="lnz")
                    nc.scalar.activation(
                        out=lnz[:], in_=zp[:], func=mybir.ActivationFunctionType.Ln
                    )
                    rz = spool.tile([128, 2 * SUB], f32, tag="rz")
                    if act_rz:
                        nc.scalar.activation(
                            out=rz[:],
                            in_=lnz[:],
                            func=mybir.ActivationFunctionType.Exp,
                            scale=-1.0,
                        )
                    else:
                        nc.vector.reciprocal_approx_fast(out=rz[:], in_=zp[:])
                    for ph in range(2):
                        u = 2 * u2 + ph
                        pair = c0 // (2 * SUB) + u
                        msl = mk[:, u * SUB : (u + 1) * SUB]
                        phs = slice(ph * SUB, (ph + 1) * SUB)
                        urz = spool.tile([128, SUB], f16, tag="urz")
                        nc.vector.scalar_tensor_tensor(
                            out=urz[:],
                            in0=ups[ph][:],
                            scalar=1.0,
                            in1=rz[:, phs],
                            op0=mybir.AluOpType.mult,
                            op1=mybir.AluOpType.mult,
                        )
                        # ent = lnZ - U/Z, then one masked accumulation
                        ent = spool.tile([128, SUB], f16, tag="ent")
                        nc.vector.tensor_tensor(
                            ent[:], lnz[:, phs], urz[:], mybir.AluOpType.subtract
                        )
                        d1 = spool.tile([128, SUB], f16, tag="d1")
                        nc.vector.scalar_tensor_tensor(
                            out=d1[:],
                            in0=ent[:],
                            scalar=1.0,
                            in1=msl,
                            op0=mybir.AluOpType.mult,
                            op1=mybir.AluOpType.mult,
                            accum_out=parts1[:, pair : pair + 1],
                        )

            nc.sync.dma_start(out=parts1_out, in_=parts1[:])
            nc.sync.dma_start(out=parts2_out, in_=parts1[:])

    nc.compile()
    _CACHE["nc"] = nc
    return nc


def _prep_inputs(prototype_activations, target_labels, proto_idx):
    acts = np.asarray(prototype_activations, dtype=np.float32)
    labels = np.asarray(target_labels)
    pidx = np.asarray(proto_idx)

    expected = np.arange(S * C * K, dtype=np.int64).reshape(S, C, K)
    if not np.array_equal(pidx.astype(np.int64), expected):
        # general (slow) fallback: permute proto columns on host
        acts = np.ascontiguousarray(acts[..., pidx.reshape(-1)])

    gms = _group_matrices()
    in_maps = []
    for b in range(B):
        x16 = acts[b].astype(np.float16)  # [N, 320]
        # [640, M]: row q*320+P = proto P of pixels n = 2m+q
        at = np.ascontiguousarray(
            x16.reshape(M, 2, PP).transpose(1, 2, 0)
        ).reshape(NT, 128, M)

        lab = labels[b].astype(np.int32)
        # L[q, u, blk, x] = label of pixel n = 2*(512*(2u+blk)+x) + q
        L = np.ascontiguousarray(lab.reshape(M, 2).T).reshape(2, NPAIR, 2, SUB)
        eq = L[:, :, :, :, None] == np.arange(1, C + 1, dtype=np.int32)
        # maskh[blk*64 + q*32 + s*8 + c, u*512 + x]
        mh = np.broadcast_to(
            eq.transpose(2, 0, 4, 1, 3)[:, :, None, :, :, :],
            (2, 2, S, C, NPAIR, SUB),
        ).astype(np.float16)
        in_maps.append(
            {
                "acts_t": at,
                "maskh": np.ascontiguousarray(mh).reshape(128, M // 2),
                "gmat": gms,
            }
        )
    return in_maps, labels


def _combine(parts_list, labels):
    """parts_list: per-core (parts1 [128, 32], parts2 [128, 32]) f32.
    Row = blk*64 + q*32 + s*8 + c, col = pair index."""
    num = np.zeros((B, S, C), dtype=np.float64)
    cnt = np.zeros((B, C), dtype=np.int64)
    for b, (p1, p2) in enumerate(parts_list):
        d = p1.astype(np.float64).sum(axis=1)
        num[b] = d.reshape(2, 2, S, C).sum(axis=(0, 1))
        lab = np.asarray(labels[b]).astype(np.int64)
        cnt[b] = np.bincount(lab, minlength=C + 1)[1 : C + 1]
    num /= np.log(K)
    present = cnt > 0
    mean_ent = num / np.maximum(cnt, 1)[:, None, :]
    n_entries = float(present.sum() * S)
    total = float((mean_ent * present[:, None, :]).sum())
    if n_entries > 0:
        return np.float32(total / max(n_entries, 1.0))
    return np.float32(0.0)


def kernel(prototype_activations, target_labels, proto_idx, _trace=False, _tmpdir=None):
    nc = _build()
    in_maps, labels = _prep_inputs(prototype_activations, target_labels, proto_idx)
    res = run_bass_kernel_spmd(
        nc, in_maps, list(range(NCORES)), trace=_trace, tmpdir=_tmpdir
    )
    parts_list = [
        (res.results[i]["parts1"], res.results[i]["parts2"]) for i in range(NCORES)
    ]
    out = _combine(parts_list, labels)
    if _trace:
        return out, res
    return out
